# revision 1
# baseline (speedup 1.0000x reference)
"""GuidedFilter (r=15, eps=0.5) Trainium2 Bass kernel.

Full inputs: guide, input_map [16,1,1024,1024] f32. Data-parallel over 8
NeuronCores (2 images/core). Per image:
  box(x) = Hpass(Vpass(x)) with 31-tap window sums, reflect padding.
  - H direction (free axis): one tensor_tensor_scan per row-tile:
        state = (x[w+15] + state) - x[w-16]  over a mirrored-pad buffer.
  - V direction (partition axis): PE band matmuls with constant bf16
    weights (reflect folded into the band blocks), fp32 PSUM accumulate.
  Elementwise chain fused with PSUM evacuation via scalar_tensor_tensor.
"""

import numpy as np
import ml_dtypes

R = 15
K = 2 * R + 1  # 31
EPS = 0.5
NORM = 1.0 / (K * K)  # 1/961

_CACHE = {}


def _build_band_weights(Hc, NT):
    """Wf[k, m] = weight of input row k in output row m's reflect window."""
    Wf = np.zeros((Hc, Hc), np.float32)
    for m in range(Hc):
        for t in range(m - R, m + R + 1):
            k = t
            if k < 0:
                k = -k
            if k > Hc - 1:
                k = 2 * (Hc - 1) - k
            Wf[k, m] += 1.0
    # Pack per out-tile j into [128, 3*128]:
    #   cols 0:128   = center block  (in-tile j,   K=128)
    #   cols 128:256 = top edge      (in-tile j-1 rows 113:128, K=15, rows 0:15)
    #   cols 256:384 = bottom edge   (in-tile j+1 rows 0:15,    K=15, rows 0:15)
    wv = np.zeros((NT, 128, 384), np.float32)
    for j in range(NT):
        r0 = j * 128
        wv[j, :, 0:128] = Wf[r0 : r0 + 128, r0 : r0 + 128]
        if j > 0:
            wv[j, 64:128, 128:256] = Wf[r0 - 64 : r0, r0 : r0 + 128]
        if j < NT - 1:
            wv[j, 0:15, 256:384] = Wf[r0 + 128 : r0 + 143, r0 : r0 + 128]
    return wv.astype(ml_dtypes.bfloat16)


def build_nc(n_img, Hc, Wc):
    """Build the Bass module for one core processing n_img images of [Hc, Wc]."""
    import concourse.bass as bass
    import concourse.tile as tile
    from concourse import bacc, mybir

    P = 128
    NT = Hc // P
    PW = Wc + 32          # padded width; interior at cols 16..16+Wc
    CH = min(512, Wc)     # psum chunk width
    NC_ = Wc // CH        # chunks per tile
    f32 = mybir.dt.float32
    bf16 = mybir.dt.bfloat16
    AX = mybir.AxisListType.X
    OP = mybir.AluOpType
    AF = mybir.ActivationFunctionType

    nc = bacc.Bacc("TRN2", target_bir_lowering=False, debug=False)
    g_dram = nc.dram_tensor("guide", [n_img, Hc, Wc], f32, kind="ExternalInput")
    p_dram = nc.dram_tensor("input_map", [n_img, Hc, Wc], f32, kind="ExternalInput")
    wv_dram = nc.dram_tensor("wv", [NT, 128, 384], bf16, kind="ExternalInput")
    o_dram = nc.dram_tensor("out", [n_img, Hc, Wc], f32, kind="ExternalOutput")
    gap, pap, wap, oap = g_dram.ap(), p_dram.ap(), wv_dram.ap(), o_dram.ap()

    with tile.TileContext(nc) as tc:
        wpool = tc.alloc_tile_pool(name="wv", bufs=1)
        wv_sb = []
        for j in range(NT):
            wt = wpool.tile([128, 384], bf16, tag=f"wv{j}", name=f"wv{j}")
            nc.sync.dma_start(wt[:], wap[j])
            wv_sb.append(wt)

        xpi_pool = tc.alloc_tile_pool(name="xpi", bufs=NT + 1)   # guide, image-long
        xpp_pool = tc.alloc_tile_pool(name="xpp", bufs=4)
        xpm_pool = tc.alloc_tile_pool(name="xpm", bufs=2)        # Ip & II pads
        h_pool = tc.alloc_tile_pool(name="hx", bufs=4)          # 4 tensors x 4
        cf_pool = tc.alloc_tile_pool(name="cf", bufs=3)         # coeff transients
        ab_pool = tc.alloc_tile_pool(name="ab", bufs=4)          # xp_a, xp_b pads
        hab_pool = tc.alloc_tile_pool(name="hab", bufs=4)        # ha, hb
        o_pool = tc.alloc_tile_pool(name="o", bufs=4)
        ps_pool = tc.alloc_tile_pool(name="ps", bufs=1, space="PSUM")
        psab_pool = tc.alloc_tile_pool(name="psab", bufs=2, space="PSUM")

        def mirrors(xp, eng=None):
            # left: cols 0:16 <- interior cols 32..17 (x[16..1]); right symmetric.
            c0 = 16 + Wc
            nc.scalar.copy(xp[:, 0:16], xp[:, 32:16:-1])
            nc.scalar.copy(xp[:, c0 : c0 + 15], xp[:, c0 - 2 : c0 - 17 : -1])

        def hscan(xp, out, dtag, pool):
            init = pool.tile([128, 1], f32, tag=f"init{dtag}", name=f"init{dtag}")
            nc.vector.reduce_sum(init[:], xp[:, 0:31], axis=AX)
            nc.vector.tensor_tensor_scan(
                out[:], xp[:, 31 : 31 + Wc], xp[:, 0:Wc], init[:],
                op0=OP.add, op1=OP.subtract,
            )

        def vpass(psum, hsrc, j, c):
            """psum[128, CH] = band-weighted column sums of hsrc tiles."""
            lo, hi = c * CH, (c + 1) * CH
            last_center = (j == 0 or hsrc[j - 1] is None) and (
                j == NT - 1 or hsrc[j + 1] is None
            )
            nc.tensor.matmul(
                psum[:], wv_sb[j][:, 0:128], hsrc[j][:, lo:hi],
                start=True, stop=last_center,
            )
            if j > 0 and hsrc[j - 1] is not None:
                nc.tensor.matmul(
                    psum[:], wv_sb[j][64:128, 128:256], hsrc[j - 1][64:128, lo:hi],
                    start=False, stop=(j == NT - 1 or hsrc[j + 1] is None),
                )
            if j < NT - 1 and hsrc[j + 1] is not None:
                nc.tensor.matmul(
                    psum[:], wv_sb[j][0:15, 256:384], hsrc[j + 1][0:15, lo:hi],
                    start=False, stop=True,
                )

        for img in range(n_img):
            xpI = [None] * NT
            hI = [None] * NT
            hp = [None] * NT
            hIp = [None] * NT
            hII = [None] * NT
            xpa = [None] * NT
            xpb = [None] * NT
            ha = [None] * NT
            hb = [None] * NT

            def stageAB(j):
                xpI[j] = xpi_pool.tile([128, PW], f32, tag="xpI", name="xpI")
                xpP = xpp_pool.tile([128, PW], f32, tag="xpP", name="xpP")
                nc.sync.dma_start(xpI[j][:, 16 : 16 + Wc], gap[img, j * 128 : (j + 1) * 128, :])
                nc.sync.dma_start(xpP[:, 16 : 16 + Wc], pap[img, j * 128 : (j + 1) * 128, :])
                mirrors(xpI[j], nc.vector)
                mirrors(xpP, nc.vector)
                xpIp = xpm_pool.tile([128, PW], bf16, tag="xpIp", name="xpIp")
                xpII = xpm_pool.tile([128, PW], bf16, tag="xpII", name="xpII")
                nc.vector.tensor_mul(
                    xpIp[:, 16 : 16 + Wc], xpI[j][:, 16 : 16 + Wc], xpP[:, 16 : 16 + Wc]
                )
                nc.scalar.activation(
                    xpII[:, 16 : 16 + Wc], xpI[j][:, 16 : 16 + Wc], AF.Square
                )
                mirrors(xpIp, nc.vector)
                mirrors(xpII, nc.vector)
                hI[j] = h_pool.tile([128, Wc], bf16, tag="hI", name="hI")
                hp[j] = h_pool.tile([128, Wc], bf16, tag="hp", name="hp")
                hIp[j] = h_pool.tile([128, Wc], bf16, tag="hIp", name="hIp")
                hII[j] = h_pool.tile([128, Wc], bf16, tag="hII", name="hII")
                hscan(xpI[j], hI[j], "I", cf_pool)
                hscan(xpP, hp[j], "p", cf_pool)
                hscan(xpIp, hIp[j], "Ip", cf_pool)
                hscan(xpII, hII[j], "II", cf_pool)

            def stageCD(j):
                xpa[j] = ab_pool.tile([128, PW], bf16, tag="xpa", name="xpa")
                xpb[j] = ab_pool.tile([128, PW], bf16, tag="xpb", name="xpb")
                for c in range(NC_):
                    mI = ps_pool.tile([128, CH], f32, tag="psA", name="psA")
                    mp = ps_pool.tile([128, CH], f32, tag="psB", name="psB")
                    mIp = ps_pool.tile([128, CH], f32, tag="psC", name="psC")
                    mII = ps_pool.tile([128, CH], f32, tag="psD", name="psD")
                    vpass(mI, hI, j, c)
                    vpass(mp, hp, j, c)
                    vpass(mIp, hIp, j, c)
                    vpass(mII, hII, j, c)
                    mI_s = cf_pool.tile([128, CH], f32, tag="mI_s", name="mI_s")
                    nc.scalar.activation(mI_s[:], mI[:], AF.Copy, scale=NORM)
                    prod = cf_pool.tile([128, CH], f32, tag="prod", name="prod")
                    nc.vector.scalar_tensor_tensor(
                        prod[:], mp[:], NORM, mI_s[:], op0=OP.mult, op1=OP.mult
                    )
                    cov = cf_pool.tile([128, CH], f32, tag="cov", name="cov")
                    nc.vector.scalar_tensor_tensor(
                        cov[:], mIp[:], NORM, prod[:], op0=OP.mult, op1=OP.subtract
                    )
                    sqI = cf_pool.tile([128, CH], f32, tag="sqI", name="sqI")
                    nc.scalar.activation(sqI[:], mI_s[:], AF.Square)
                    d = cf_pool.tile([128, CH], f32, tag="d", name="d")
                    nc.vector.scalar_tensor_tensor(
                        d[:], mII[:], NORM, sqI[:], op0=OP.mult, op1=OP.subtract
                    )
                    d2 = cf_pool.tile([128, CH], f32, tag="d2", name="d2")
                    nc.scalar.activation(d2[:], d[:], AF.Copy, bias=EPS)
                    r = cf_pool.tile([128, CH], f32, tag="r", name="r")
                    nc.vector.reciprocal_approx_fast(out=r[:], in_=d[:] if False else d2[:])
                    lo = 16 + c * CH
                    av = xpa[j][:, lo : lo + CH]
                    nc.vector.tensor_mul(av, cov[:], r[:])
                    t = cf_pool.tile([128, CH], f32, tag="t", name="t")
                    nc.vector.tensor_mul(t[:], av, mI_s[:])
                    nc.vector.scalar_tensor_tensor(
                        xpb[j][:, lo : lo + CH], mp[:], NORM, t[:],
                        op0=OP.mult, op1=OP.subtract,
                    )
                mirrors(xpa[j], nc.vector)
                mirrors(xpb[j], nc.vector)
                ha[j] = hab_pool.tile([128, Wc], bf16, tag="ha", name="ha")
                hb[j] = hab_pool.tile([128, Wc], bf16, tag="hb", name="hb")
                hscan(xpa[j], ha[j], "a", cf_pool)
                hscan(xpb[j], hb[j], "b", cf_pool)

            def stageF(j):
                for c in range(NC_):
                    ma = psab_pool.tile([128, CH], f32, tag="psa", name="psa")
                    mb = psab_pool.tile([128, CH], f32, tag="psb", name="psb")
                    vpass(ma, ha, j, c)
                    vpass(mb, hb, j, c)
                    o1 = o_pool.tile([128, CH], f32, tag="o1", name="o1")
                    nc.vector.scalar_tensor_tensor(
                        o1[:], ma[:], NORM, xpI[j][:, 16 + c * CH : 16 + (c + 1) * CH],
                        op0=OP.mult, op1=OP.mult,
                    )
                    o2 = o_pool.tile([128, CH], f32, tag="o2", name="o2")
                    nc.vector.scalar_tensor_tensor(
                        o2[:], mb[:], NORM, o1[:], op0=OP.mult, op1=OP.add
                    )
                    nc.sync.dma_start(
                        oap[img, j * 128 : (j + 1) * 128, c * CH : (c + 1) * CH], o2[:]
                    )

            # software-pipelined emission: AB leads CD by 2 tiles, F lags CD by 1
            stageAB(0)
            if NT > 1:
                stageAB(1)
            for j in range(NT):
                if j + 2 < NT:
                    stageAB(j + 2)
                stageCD(j)
                if j >= 1:
                    stageF(j - 1)
            stageF(NT - 1)

        for _pool in (psab_pool, ps_pool, o_pool, hab_pool, ab_pool, cf_pool,
                      h_pool, xpm_pool, xpp_pool, xpi_pool, wpool):
            _pool.release()

    nc.compile()
    return nc


def _get_nc(n_img, Hc, Wc):
    key = (n_img, Hc, Wc)
    if key not in _CACHE:
        _CACHE[key] = build_nc(n_img, Hc, Wc)
    return _CACHE[key]


def kernel(guide, input_map):
    from concourse.bass_utils import run_bass_kernel_spmd

    B, C, Hc, Wc = guide.shape
    n_cores = 8
    n_img = B // n_cores
    g = np.ascontiguousarray(guide.reshape(B, Hc, Wc), dtype=np.float32)
    p = np.ascontiguousarray(input_map.reshape(B, Hc, Wc), dtype=np.float32)
    wv = _build_band_weights(Hc, Hc // 128)
    nc = _get_nc(n_img, Hc, Wc)
    in_maps = [
        {
            "guide": g[i * n_img : (i + 1) * n_img],
            "input_map": p[i * n_img : (i + 1) * n_img],
            "wv": wv,
        }
        for i in range(n_cores)
    ]
    res = run_bass_kernel_spmd(nc, in_maps, core_ids=list(range(n_cores)))
    out = np.concatenate([res.results[i]["out"] for i in range(n_cores)], axis=0)
    return out.reshape(B, C, Hc, Wc).astype(np.float32)



# revision 8
# speedup vs baseline: 1.0926x; 1.0926x over previous
"""GuidedFilter (r=15, eps=0.5) Trainium2 Bass kernel, v2.

Full inputs: guide, input_map [16,1,1024,1024] f32. Data-parallel over 8
NeuronCores (2 images/core). Per image, per box filter the order is
V-pass first, then H-pass:
  - V direction (partition axis): PE band matmuls. Round 1 uses fp8e4m3
    inputs with DoubleRow perf mode (2 k-subtiles per matmul: center and
    edge band blocks fused, 0.5 cyc/row). Round 2 (a, b) runs bf16.
  - PSUM evacuation on the Act engine with the 1/961 box normalization
    folded into the copy scale, written into 4-segment mirror-padded
    bf16 buffers.
  - H direction (free axis): ONE tensor_tensor_scan per tile covering
    all segments back-to-back (running 31-window sum telescopes exactly
    across the inter-segment padding).
The elementwise chain runs as packed-bf16 scalar_tensor_tensor ops (4x
DVE mode); 1/(var+eps) is a linear minimax seed + one Newton step with
the sign folded away. Final combine (mean_a*I + mean_b) runs on the
GPSIMD engine. Host pre-stages fp8/bf16 inputs in a tile-transposed
layout and converts the bf16 output back to f32.
"""

import numpy as np
import ml_dtypes

R = 15
K = 2 * R + 1  # 31
EPS = 0.5
ALPHA = 1.0 / (K * K)  # evac scale: PSUM V-sums -> means after the H scan

# minimax linear fit of 1/d on [DLO, DHI]; d = var + EPS
DLO, DHI = 0.47, 0.85
_B = 1.0 / (DLO * DHI)
_A = 0.5 * ((DLO + DHI) / (DLO * DHI) + 2.0 / np.sqrt(DLO * DHI))
_A1 = _A - _B * EPS  # r0 = _A1 - _B * var

_CACHE = {}


def _band_blocks(Hc):
    """Wf[k, m]: weight of input row k in output row m's reflect window."""
    Wf = np.zeros((Hc, Hc), np.float32)
    for m in range(Hc):
        for t in range(m - R, m + R + 1):
            k = t
            if k < 0:
                k = -k
            if k > Hc - 1:
                k = 2 * (Hc - 1) - k
            Wf[k, m] += 1.0
    return Wf


def _build_weights(Hc, NT):
    Wf = _band_blocks(Hc)
    C = []
    T = []  # T[j]: rows of tile j-1 (placed at partitions 113:128)
    B = []  # B[j]: rows of tile j+1 (placed at partitions 0:15)
    for j in range(NT):
        r0 = j * 128
        C.append(Wf[r0 : r0 + 128, r0 : r0 + 128])
        Tj = np.zeros((128, 128), np.float32)
        if j > 0:
            Tj[128 - R :, :] = Wf[r0 - R : r0, r0 : r0 + 128]
        T.append(Tj)
        Bj = np.zeros((128, 128), np.float32)
        if j < NT - 1:
            Bj[:R, :] = Wf[r0 + 128 : r0 + 128 + R, r0 : r0 + 128]
        B.append(Bj)

    # fp8 DR weights, [128, n_mm, 2, 128]:
    #   j=0         -> [C_0 | B_0]          rhs (x0, x1)
    #   interior j  -> [T_j | B_j]          rhs (x_{j-1}, x_{j+1})
    #                  [0   | C_j]          rhs (x_{j-1}, x_j)
    #   j=NT-1      -> [T | C]              rhs (x_{NT-2}, x_{NT-1})
    w8_list = []
    idx = {}
    for j in range(NT):
        if j == 0:
            idx[j] = [len(w8_list)]
            w8_list.append(np.stack([C[0], B[0]]))
        elif j == NT - 1:
            idx[j] = [len(w8_list)]
            w8_list.append(np.stack([T[j], C[j]]))
        else:
            idx[j] = [len(w8_list), len(w8_list) + 1]
            w8_list.append(np.stack([T[j], B[j]]))
            w8_list.append(np.stack([np.zeros((128, 128), np.float32), C[j]]))
    w8 = np.stack(w8_list)  # [n_mm, 2, 128k, 128m]
    w8 = np.ascontiguousarray(w8.transpose(2, 0, 1, 3))  # [128k, n_mm, 2, 128m]
    w8 = w8.astype(ml_dtypes.float8_e4m3)

    # bf16 round-2 weights [128, NT, 384]: center | top | bottom
    w16 = np.zeros((NT, 128, 384), np.float32)
    for j in range(NT):
        w16[j, :, 0:128] = C[j]
        w16[j, :, 128:256] = T[j]
        w16[j, :, 256:384] = B[j]
    w16 = np.ascontiguousarray(w16.transpose(1, 0, 2)).astype(ml_dtypes.bfloat16)
    return w8, idx, w16


def build_nc(n_img, Hc, Wc):
    import concourse.bass as bass
    import concourse.tile as tile
    from concourse import bacc, mybir

    P = 128
    NT = Hc // P          # 8 row tiles
    SW = Wc + 32          # padded segment width: 16 | Wc | 16
    CH = 512              # psum chunk width
    NC_ = Wc // CH
    f32 = mybir.dt.float32
    bf16 = mybir.dt.bfloat16
    fp8 = mybir.dt.float8e4
    AX = mybir.AxisListType.X
    OP = mybir.AluOpType
    AF = mybir.ActivationFunctionType
    DR = mybir.MatmulPerfMode.DoubleRow

    w8_np, w8_idx, _ = _build_weights(Hc, NT)
    NMM = w8_np.shape[1]

    nc = bacc.Bacc("TRN2", target_bir_lowering=False, debug=False)
    dI8 = nc.dram_tensor("I8", [n_img, P, NT, Wc], fp8, kind="ExternalInput")
    dp8 = nc.dram_tensor("p8", [n_img, P, NT, Wc], fp8, kind="ExternalInput")
    dIp8 = nc.dram_tensor("Ip8", [n_img, P, NT, Wc], fp8, kind="ExternalInput")
    dII8 = nc.dram_tensor("II8", [n_img, P, NT, Wc], fp8, kind="ExternalInput")
    dI16 = nc.dram_tensor("I16", [n_img, P, NT, Wc], bf16, kind="ExternalInput")
    dw8 = nc.dram_tensor("w8", [P, NMM, 2, 128], fp8, kind="ExternalInput")
    dw16 = nc.dram_tensor("w16", [P, NT, 384], bf16, kind="ExternalInput")
    dout = nc.dram_tensor("out", [n_img, P, NT, Wc], bf16, kind="ExternalOutput")

    with tile.TileContext(nc) as tc:
        wpool = tc.alloc_tile_pool(name="w", bufs=1)
        xpool = tc.alloc_tile_pool(name="x", bufs=1)
        i16pool = tc.alloc_tile_pool(name="i16", bufs=2)
        opool = tc.alloc_tile_pool(name="o", bufs=1)
        abpool = tc.alloc_tile_pool(name="ab", bufs=1)
        vs1pool = tc.alloc_tile_pool(name="vs1", bufs=2)
        so1pool = tc.alloc_tile_pool(name="so1", bufs=2)
        vs2pool = tc.alloc_tile_pool(name="vs2", bufs=2)
        so2pool = tc.alloc_tile_pool(name="so2", bufs=2)
        cpool = tc.alloc_tile_pool(name="c", bufs=1)
        ps1 = tc.alloc_tile_pool(name="ps1", bufs=1, space="PSUM")
        ps2 = tc.alloc_tile_pool(name="ps2", bufs=1, space="PSUM")

        w8sb = wpool.tile([P, NMM, 2, 128], fp8, tag="w8", name="w8sb")
        nc.sync.dma_start(w8sb[:], dw8.ap())
        w16sb = wpool.tile([P, NT, 384], bf16, tag="w16", name="w16sb")
        nc.sync.dma_start(w16sb[:], dw16.ap())

        def mirrors(vs):
            # left pads <- interior cols 17..32 reversed; right pads (incl.
            # slack col) <- interior cols 1023..1038 reversed. All segs at once.
            nc.vector.tensor_copy(vs[:, :, 0:16], vs[:, :, 32:16:-1])
            nc.vector.tensor_copy(vs[:, :, SW - 16 : SW], vs[:, :, SW - 18 : SW - 34 : -1])

        for img in range(n_img):
            xI8 = xpool.tile([P, NT, Wc], fp8, tag="xI8", name="xI8")
            xp8 = xpool.tile([P, NT, Wc], fp8, tag="xp8", name="xp8")
            xIp8 = xpool.tile([P, NT, Wc], fp8, tag="xIp8", name="xIp8")
            xII8 = xpool.tile([P, NT, Wc], fp8, tag="xII8", name="xII8")
            i16 = i16pool.tile([P, NT, Wc], bf16, tag="i16", name="i16")
            nc.sync.dma_start(xI8[:], dI8.ap()[img])
            nc.sync.dma_start(xp8[:], dp8.ap()[img])
            nc.sync.dma_start(xIp8[:], dIp8.ap()[img])
            nc.sync.dma_start(xII8[:], dII8.ap()[img])
            nc.sync.dma_start(i16[:], dI16.ap()[img])
            outB = opool.tile([P, NT, Wc], bf16, tag="outB", name="outB")
            aB = abpool.tile([P, NT, Wc], bf16, tag="aB", name="aB")
            bB = abpool.tile([P, NT, Wc], bf16, tag="bB", name="bB")
            so1s = [None] * NT
            so2s = [None] * NT

            X8 = (("I", xI8), ("p", xp8), ("Ip", xIp8), ("II", xII8))

            def stage1(j):
                vs1 = vs1pool.tile([P, 4, SW], bf16, tag="vs1", name="vs1")
                qs = {}
                for tagx, _ in X8:
                    qs[tagx] = ps1.tile([P, CH], f32, tag=f"q{tagx}", name=f"q{tagx}")
                mms = w8_idx[j]
                for c in range(NC_):
                    lo = c * CH
                    for s, (tagx, xt) in enumerate(X8):
                        q = qs[tagx]
                        for mi, mm in enumerate(mms):
                            if j == 0:
                                rhs = xt[:, 0:2, lo : lo + CH]
                            elif j == NT - 1:
                                rhs = xt[:, NT - 2 : NT, lo : lo + CH]
                            elif mi == 0:
                                rhs = xt[:, j - 1 : j + 2 : 2, lo : lo + CH]
                            else:
                                rhs = xt[:, j - 1 : j + 1, lo : lo + CH]
                            nc.tensor.matmul(
                                q[:], w8sb[:, mm], rhs,
                                start=(mi == 0), stop=(mi == len(mms) - 1),
                                perf_mode=DR,
                            )
                        nc.scalar.activation(
                            vs1[:, s, 16 + lo : 16 + lo + CH], q[:], AF.Copy,
                            scale=ALPHA,
                        )
                mirrors(vs1)
                flat = vs1[:].rearrange("p s w -> p (s w)")
                L = 4 * SW
                init = cpool.tile([P, 1], f32, tag="init1", name="init1", bufs=2)
                nc.vector.reduce_sum(init[:], flat[:, 0:K], axis=AX)
                so1 = so1pool.tile([P, L], bf16, tag="so1", name="so1")
                nc.vector.tensor_tensor_scan(
                    so1[:, 0 : L - K], flat[:, K:L], flat[:, 0 : L - K], init[:],
                    op0=OP.add, op1=OP.subtract,
                )
                so1s[j] = so1

            def stage2(j):
                so1 = so1s[j]
                sI = so1[:, 0:Wc]
                sp = so1[:, SW : SW + Wc]
                sIp = so1[:, 2 * SW : 2 * SW + Wc]
                sII = so1[:, 3 * SW : 3 * SW + Wc]
                prod = cpool.tile([P, Wc], bf16, tag="tmpA", name="prod")
                nc.vector.scalar_tensor_tensor(prod[:], sI, 1.0, sp, op0=OP.mult, op1=OP.mult)
                covs = cpool.tile([P, Wc], bf16, tag="tmpB", name="covs")
                nc.vector.scalar_tensor_tensor(covs[:], sIp, 1.0, prod[:], op0=OP.mult, op1=OP.subtract)
                sq = cpool.tile([P, Wc], bf16, tag="tmpC", name="sq")
                nc.scalar.activation(sq[:], sI, AF.Square)
                dn = cpool.tile([P, Wc], bf16, tag="tmpA", name="dn")
                nc.vector.scalar_tensor_tensor(dn[:], sII, 1.0, sq[:], op0=OP.mult, op1=OP.subtract)
                # -r ~ -1/(dn + EPS): r0n = B*dn - A1; ng = (dn+EPS)*r0n;
                # rcn = (ng+2)*r0n = -(2 - d*r0)*r0
                r0n = cpool.tile([P, Wc], bf16, tag="tmpC", name="r0n")
                nc.vector.tensor_scalar(r0n[:], dn[:], _B, -_A1, op0=OP.mult, op1=OP.add)
                ng = cpool.tile([P, Wc], bf16, tag="tmpD", name="ng")
                nc.vector.scalar_tensor_tensor(ng[:], dn[:], EPS, r0n[:], op0=OP.add, op1=OP.mult)
                rcn = cpool.tile([P, Wc], bf16, tag="tmpA", name="rcn")
                nc.vector.scalar_tensor_tensor(rcn[:], ng[:], 2.0, r0n[:], op0=OP.add, op1=OP.mult)
                av = aB[:, j, :]
                nc.vector.scalar_tensor_tensor(av, covs[:], -1.0, rcn[:], op0=OP.mult, op1=OP.mult)
                t = cpool.tile([P, Wc], bf16, tag="tmpC", name="t")
                nc.vector.scalar_tensor_tensor(t[:], av, 1.0, sI, op0=OP.mult, op1=OP.mult)
                nc.vector.scalar_tensor_tensor(bB[:, j, :], sp, 1.0, t[:], op0=OP.mult, op1=OP.subtract)

            def stage3(j):
                vs2 = vs2pool.tile([P, 2, SW], bf16, tag="vs2", name="vs2")
                for s, ab in enumerate((aB, bB)):
                    q2 = ps2.tile([P, Wc], f32, tag=f"q2{s}", name=f"q2{s}")
                    for c in range(NC_):
                        lo = c * CH
                        nc.tensor.matmul(
                            q2[:, lo : lo + CH], w16sb[:, j, 0:128],
                            ab[:, j, lo : lo + CH],
                            start=True, stop=(j == 0 and j == NT - 1),
                        )
                        if j > 0:
                            nc.tensor.matmul(
                                q2[:, lo : lo + CH],
                                w16sb[64:128, j, 128:256],
                                ab[64:128, j - 1, lo : lo + CH],
                                start=False, stop=(j == NT - 1),
                            )
                        if j < NT - 1:
                            nc.tensor.matmul(
                                q2[:, lo : lo + CH], w16sb[0:32, j, 256:384],
                                ab[0:32, j + 1, lo : lo + CH],
                                start=False, stop=True,
                            )
                    nc.scalar.activation(vs2[:, s, 16 : 16 + Wc], q2[:], AF.Copy, scale=ALPHA)
                mirrors(vs2)
                flat = vs2[:].rearrange("p s w -> p (s w)")
                L = 2 * SW
                init = cpool.tile([P, 1], f32, tag="init2", name="init2", bufs=2)
                nc.vector.reduce_sum(init[:], flat[:, 0:K], axis=AX)
                so2 = so2pool.tile([P, L], bf16, tag="so2", name="so2")
                nc.vector.tensor_tensor_scan(
                    so2[:, 0 : L - K], flat[:, K:L], flat[:, 0 : L - K], init[:],
                    op0=OP.add, op1=OP.subtract,
                )
                so2s[j] = so2

            def stage4(j):
                so2 = so2s[j]
                sa = so2[:, 0:Wc]
                sb = so2[:, SW : SW + Wc]
                o1 = cpool.tile([P, Wc], bf16, tag="o1", name="o1", bufs=2)
                nc.gpsimd.tensor_tensor(o1[:], sa, i16[:, j, :], op=OP.mult)
                nc.gpsimd.tensor_tensor(outB[:, j, :], sb, o1[:], op=OP.add)

            stage1(0)
            stage1(1)
            for j in range(NT):
                stage2(j)
                if j + 2 < NT:
                    stage1(j + 2)
                if j >= 1:
                    stage3(j - 1)
                    stage4(j - 1)
            stage3(NT - 1)
            stage4(NT - 1)
            nc.sync.dma_start(dout.ap()[img], outB[:])

        for p_ in (ps2, ps1, cpool, so2pool, vs2pool, so1pool, vs1pool,
                   abpool, opool, i16pool, xpool, wpool):
            p_.release()

    nc.compile()
    return nc


def _get_nc(n_img, Hc, Wc):
    key = (n_img, Hc, Wc)
    if key not in _CACHE:
        _CACHE[key] = build_nc(n_img, Hc, Wc)
    return _CACHE[key]


def _to_tiled(a, NT=8, P=128):
    # [n, H, W] -> [n, P, NT, W] with row r = j*P + p stored at [p, j]
    n, H, W = a.shape
    return np.ascontiguousarray(a.reshape(n, NT, P, W).transpose(0, 2, 1, 3))


def _from_tiled(a):
    # [n, P, NT, W] -> [n, H, W]
    n, P_, NT, W = a.shape
    return a.transpose(0, 2, 1, 3).reshape(n, NT * P_, W)


def kernel(guide, input_map):
    from concourse.bass_utils import run_bass_kernel_spmd

    B, C, Hc, Wc = guide.shape
    n_cores = 8
    n_img = B // n_cores
    NT = Hc // 128
    g = np.asarray(guide, dtype=np.float32).reshape(B, Hc, Wc)
    p = np.asarray(input_map, dtype=np.float32).reshape(B, Hc, Wc)

    f8 = ml_dtypes.float8_e4m3
    b16 = ml_dtypes.bfloat16
    I8 = _to_tiled(g).astype(f8)
    p8 = _to_tiled(p).astype(f8)
    Ip8 = _to_tiled(g * p).astype(f8)
    II8 = _to_tiled(g * g).astype(f8)
    I16 = _to_tiled(g).astype(b16)

    w8, _, w16 = _build_weights(Hc, NT)
    nc = _get_nc(n_img, Hc, Wc)
    in_maps = [
        {
            "I8": I8[i * n_img : (i + 1) * n_img],
            "p8": p8[i * n_img : (i + 1) * n_img],
            "Ip8": Ip8[i * n_img : (i + 1) * n_img],
            "II8": II8[i * n_img : (i + 1) * n_img],
            "I16": I16[i * n_img : (i + 1) * n_img],
            "w8": w8,
            "w16": w16,
        }
        for i in range(n_cores)
    ]
    res = run_bass_kernel_spmd(nc, in_maps, core_ids=list(range(n_cores)))
    out = np.concatenate(
        [_from_tiled(np.asarray(res.results[i]["out"])) for i in range(n_cores)], axis=0
    )
    return out.reshape(B, C, Hc, Wc).astype(np.float32)


# revision 15
# speedup vs baseline: 1.8745x; 1.7157x over previous
"""GuidedFilter (r=15, eps=0.5) Trainium2 Bass kernel.

Full inputs: guide, input_map [16,1,1024,1024] f32. Data-parallel over 8
NeuronCores (2 images/core); both images run through ONE software
pipeline over 16 global row-tiles. Per box filter the order is V-pass
first, then H-pass:
  - V direction (partition axis): PE band matmuls. Round 1 uses fp8e4m3
    inputs with DoubleRow perf mode (2 k-subtiles per matmul fuse the
    center and edge band blocks, 0.5 cyc/row). Round 2 (a, b) runs bf16.
  - PSUM evacuation on the Act engine with the 1/961 box normalization
    folded into the copy scale, written into 4-segment mirror-padded
    bf16 buffers (fused negative-stride pad copies on DVE).
  - H direction (free axis): ONE tensor_tensor_scan per tile covering
    all segments back-to-back (the running 31-window sum telescopes
    exactly across the inter-segment padding).
The elementwise chain runs as packed-bf16 tensor_tensor ops split
across DVE (cov, var, a, b) / Act (square, linear-minimax 1/(var+eps)
seed) / GPSIMD (products, final mean_a*I + mean_b against an fp8 guide
copy). Host pre-stages I, p, I*p, I*I as fp8e4m3 in a tile-transposed
[n,128,8,1024] layout; output is written f32.
"""

import numpy as np
import ml_dtypes

R = 15
K = 2 * R + 1  # 31
EPS = 0.5
ALPHA = 1.0 / (K * K)  # evac scale: PSUM V-sums -> means after the H scan

# minimax linear fit of 1/d on [DLO, DHI]; d = var + EPS
DLO, DHI = 0.47, 0.85
_B = 1.0 / (DLO * DHI)
_A = 0.5 * ((DLO + DHI) / (DLO * DHI) + 2.0 / np.sqrt(DLO * DHI))
_A1 = _A - _B * EPS  # r0 = _A1 - _B * var

_CACHE = {}


def _band_blocks(Hc):
    """Wf[k, m]: weight of input row k in output row m's reflect window."""
    Wf = np.zeros((Hc, Hc), np.float32)
    for m in range(Hc):
        for t in range(m - R, m + R + 1):
            k = t
            if k < 0:
                k = -k
            if k > Hc - 1:
                k = 2 * (Hc - 1) - k
            Wf[k, m] += 1.0
    return Wf


def _build_weights(Hc, NT):
    Wf = _band_blocks(Hc)
    C = []
    T = []  # T[j]: rows of tile j-1 (placed at partitions 113:128)
    B = []  # B[j]: rows of tile j+1 (placed at partitions 0:15)
    for j in range(NT):
        r0 = j * 128
        C.append(Wf[r0 : r0 + 128, r0 : r0 + 128])
        Tj = np.zeros((128, 128), np.float32)
        if j > 0:
            Tj[128 - R :, :] = Wf[r0 - R : r0, r0 : r0 + 128]
        T.append(Tj)
        Bj = np.zeros((128, 128), np.float32)
        if j < NT - 1:
            Bj[:R, :] = Wf[r0 + 128 : r0 + 128 + R, r0 : r0 + 128]
        B.append(Bj)

    # fp8 DR weights, [128, n_mm, 2, 128]:
    #   j=0         -> [C_0 | B_0]          rhs (x0, x1)
    #   interior j  -> [T_j | B_j]          rhs (x_{j-1}, x_{j+1})
    #                  [0   | C_j]          rhs (x_{j-1}, x_j)
    #   j=NT-1      -> [T | C]              rhs (x_{NT-2}, x_{NT-1})
    w8_list = []
    idx = {}
    for j in range(NT):
        if j == 0:
            idx[j] = [len(w8_list)]
            w8_list.append(np.stack([C[0], B[0]]))
        elif j == NT - 1:
            idx[j] = [len(w8_list)]
            w8_list.append(np.stack([T[j], C[j]]))
        else:
            idx[j] = [len(w8_list), len(w8_list) + 1]
            w8_list.append(np.stack([T[j], B[j]]))
            w8_list.append(np.stack([np.zeros((128, 128), np.float32), C[j]]))
    w8 = np.stack(w8_list)  # [n_mm, 2, 128k, 128m]
    w8 = np.ascontiguousarray(w8.transpose(2, 0, 1, 3))  # [128k, n_mm, 2, 128m]
    w8 = w8.astype(ml_dtypes.float8_e4m3)

    # bf16 round-2 weights [128, NT, 384]: center | top | bottom
    w16 = np.zeros((NT, 128, 384), np.float32)
    for j in range(NT):
        w16[j, :, 0:128] = C[j]
        w16[j, :, 128:256] = T[j]
        w16[j, :, 256:384] = B[j]
    w16 = np.ascontiguousarray(w16.transpose(1, 0, 2)).astype(ml_dtypes.bfloat16)
    return w8, idx, w16


def build_nc(n_img, Hc, Wc):
    import concourse.bass as bass
    import concourse.tile as tile
    from concourse import bacc, mybir

    P = 128
    NT = Hc // P          # 8 row tiles per image
    NTOT = n_img * NT     # global tile count (both images, one pipeline)
    SW = Wc + 32          # padded segment width: 16 | Wc | 15 | slack
    CH = 512              # psum chunk width
    NC_ = Wc // CH
    f32 = mybir.dt.float32
    bf16 = mybir.dt.bfloat16
    fp8 = mybir.dt.float8e4
    AX = mybir.AxisListType.X
    OP = mybir.AluOpType
    AF = mybir.ActivationFunctionType
    DR = mybir.MatmulPerfMode.DoubleRow

    w8_np, w8_idx, _ = _build_weights(Hc, NT)
    NMM = w8_np.shape[1]

    nc = bacc.Bacc("TRN2", target_bir_lowering=False, debug=False)
    dI8 = nc.dram_tensor("I8", [n_img, P, NT, Wc], fp8, kind="ExternalInput")
    dp8 = nc.dram_tensor("p8", [n_img, P, NT, Wc], fp8, kind="ExternalInput")
    dIp8 = nc.dram_tensor("Ip8", [n_img, P, NT, Wc], fp8, kind="ExternalInput")
    dII8 = nc.dram_tensor("II8", [n_img, P, NT, Wc], fp8, kind="ExternalInput")
    dw8 = nc.dram_tensor("w8", [P, NMM, 2, 128], fp8, kind="ExternalInput")
    dw16 = nc.dram_tensor("w16", [P, NT, 384], bf16, kind="ExternalInput")
    dout = nc.dram_tensor("out", [n_img, P, NT, Wc], f32, kind="ExternalOutput")

    with tile.TileContext(nc) as tc:
        wpool = tc.alloc_tile_pool(name="w", bufs=1)
        xpool = tc.alloc_tile_pool(name="x", bufs=1)
        opool = tc.alloc_tile_pool(name="o", bufs=1)
        abpool = tc.alloc_tile_pool(name="ab", bufs=4)
        vs1pool = tc.alloc_tile_pool(name="vs1", bufs=3)
        so1pool = tc.alloc_tile_pool(name="so1", bufs=3)
        vs2pool = tc.alloc_tile_pool(name="vs2", bufs=2)
        so2pool = tc.alloc_tile_pool(name="so2", bufs=2)
        cpool = tc.alloc_tile_pool(name="c", bufs=1)
        ps1 = tc.alloc_tile_pool(name="ps1", bufs=1, space="PSUM")
        ps2 = tc.alloc_tile_pool(name="ps2", bufs=1, space="PSUM")

        w8sb = wpool.tile([P, NMM, 2, 128], fp8, tag="w8", name="w8sb")
        w16sb = wpool.tile([P, NT, 384], bf16, tag="w16", name="w16sb")

        def mirrors(vs):
            nc.vector.tensor_copy(vs[:, :, 0:16], vs[:, :, 32:16:-1])
            nc.vector.tensor_copy(vs[:, :, SW - 16 : SW], vs[:, :, SW - 18 : SW - 34 : -1])

        # all input DMAs upfront (quartered): image 1 loads start as soon as
        # image 0's matmuls release the buffers
        imgs = []
        wloaded = []
        nc.sync.dma_start(w8sb[:], dw8.ap())
        qh = NT // 4
        for img in range(n_img):
            xI8 = xpool.tile([P, NT, Wc], fp8, tag="xI8", name="xI8")
            xp8 = xpool.tile([P, NT, Wc], fp8, tag="xp8", name="xp8")
            xIp8 = xpool.tile([P, NT, Wc], fp8, tag="xIp8", name="xIp8")
            xII8 = xpool.tile([P, NT, Wc], fp8, tag="xII8", name="xII8")
            xI8b = xpool.tile([P, NT, Wc], fp8, tag="xI8b", name="xI8b")
            for q in range(4):
                s0, s1_ = q * qh, (q + 1) * qh
                for sb_t, dr_t in ((xI8, dI8), (xp8, dp8), (xIp8, dIp8), (xII8, dII8)):
                    nc.sync.dma_start(sb_t[:, s0:s1_, :], dr_t.ap()[img, :, s0:s1_, :])
                if len(wloaded) == 0:
                    wloaded.append(1)
                elif len(wloaded) == 1:
                    wloaded.append(1)
                    nc.sync.dma_start(w16sb[:], dw16.ap())
            for hq in range(2):
                s0, s1_ = hq * (NT // 2), (hq + 1) * (NT // 2)
                nc.sync.dma_start(xI8b[:, s0:s1_, :], dI8.ap()[img, :, s0:s1_, :])
            imgs.append((xI8, xp8, xIp8, xII8, xI8b))

        outBs = [None] * n_img
        so1s = [None] * NTOT
        so2s = [None] * NTOT
        vs1s = [None] * NTOT
        vs2s = [None] * NTOT
        aT = [None] * NTOT
        bT = [None] * NTOT
        ts_ = [None] * NTOT
        carry = {}

        def s1me(T):
            img, j = divmod(T, NT)
            xI8, xp8, xIp8, xII8, _ = imgs[img]
            vs1 = vs1pool.tile([P, 4, SW], bf16, tag="vs1", name="vs1")
            vs1s[T] = vs1
            qA = ps1.tile([P, 2 * CH], f32, tag="qA", name="qA")
            qB = ps1.tile([P, 2 * CH], f32, tag="qB", name="qB")
            mms = w8_idx[j]
            for c in range(NC_):
                lo = c * CH
                for s, xt in enumerate((xI8, xp8, xIp8, xII8)):
                    q = (qA, qB)[s // 2]
                    qlo = (s % 2) * CH
                    for mi, mm in enumerate(mms):
                        if j == 0:
                            rhs = xt[:, 0:2, lo : lo + CH]
                        elif j == NT - 1:
                            rhs = xt[:, NT - 2 : NT, lo : lo + CH]
                        elif mi == 0:
                            rhs = xt[:, j - 1 : j + 2 : 2, lo : lo + CH]
                        else:
                            rhs = xt[:, j - 1 : j + 1, lo : lo + CH]
                        nc.tensor.matmul(
                            q[:, qlo : qlo + CH], w8sb[:, mm], rhs,
                            start=(mi == 0), stop=(mi == len(mms) - 1),
                            perf_mode=DR,
                        )
                nc.scalar.activation(
                    vs1[:, 0:2, 16 + lo : 16 + lo + CH], qA[:], AF.Copy, scale=ALPHA
                )
                nc.scalar.activation(
                    vs1[:, 2:4, 16 + lo : 16 + lo + CH], qB[:], AF.Copy, scale=ALPHA
                )

        def s1s(T):
            vs1 = vs1s[T]
            L = 4 * SW
            so1 = so1pool.tile([P, L], bf16, tag="so1", name="so1")
            if T == 0:
                H2 = 2 * SW
                for h in range(2):
                    half = vs1[:, 2 * h : 2 * h + 2, :]
                    mirrors(half)
                    fl = half.rearrange("p s w -> p (s w)")
                    init = cpool.tile([P, 1], f32, tag="init1", name="init1", bufs=2)
                    nc.vector.reduce_sum(init[:], fl[:, 0:K], axis=AX)
                    nc.vector.tensor_tensor_scan(
                        so1[:, h * H2 : (h + 1) * H2 - K], fl[:, K:H2],
                        fl[:, 0 : H2 - K], init[:],
                        op0=OP.add, op1=OP.subtract,
                    )
            else:
                mirrors(vs1)
                flat = vs1[:].rearrange("p s w -> p (s w)")
                init = cpool.tile([P, 1], f32, tag="init1", name="init1", bufs=2)
                nc.vector.reduce_sum(init[:], flat[:, 0:K], axis=AX)
                nc.vector.tensor_tensor_scan(
                    so1[:, 0 : L - K], flat[:, K:L], flat[:, 0 : L - K], init[:],
                    op0=OP.add, op1=OP.subtract,
                )
            so1s[T] = so1

        def s2a(T):
            so1 = so1s[T]
            sI = so1[:, 0:Wc]
            sp = so1[:, SW : SW + Wc]
            prod = cpool.tile([P, Wc], bf16, tag="prod", name="prod")
            nc.gpsimd.tensor_mul(prod[:], sI, sp)
            sq = cpool.tile([P, Wc], bf16, tag="sq", name="sq")
            nc.scalar.activation(sq[:], sI, AF.Square)
            carry[T] = (prod, sq)

        def s2b(T):
            so1 = so1s[T]
            sIp = so1[:, 2 * SW : 2 * SW + Wc]
            sII = so1[:, 3 * SW : 3 * SW + Wc]
            prod, sq = carry[T]
            dn = cpool.tile([P, Wc], bf16, tag="dn", name="dn")
            nc.vector.tensor_sub(dn[:], sII, sq[:])
            covs = cpool.tile([P, Wc], bf16, tag="covs", name="covs")
            nc.vector.tensor_sub(covs[:], sIp, prod[:])
            r0 = cpool.tile([P, Wc], bf16, tag="r0", name="r0")
            nc.scalar.activation(r0[:], dn[:], AF.Copy, scale=-_B, bias=_A1)
            carry[T] = (covs, r0)

        def s2c(T):
            sI = so1s[T][:, 0:Wc]
            covs, r0 = carry.pop(T)
            a = abpool.tile([P, Wc], bf16, tag="aT", name="a")
            aT[T] = a
            nc.vector.tensor_mul(a[:], covs[:], r0[:])
            t = cpool.tile([P, Wc], bf16, tag="t", name="t", bufs=2)
            teng = nc.vector if T >= NTOT - 2 else nc.gpsimd
            teng.tensor_mul(t[:], a[:], sI)
            ts_[T] = t

        def s2d(T):
            sp = so1s[T][:, SW : SW + Wc]
            b = abpool.tile([P, Wc], bf16, tag="bT", name="b")
            bT[T] = b
            nc.vector.tensor_sub(b[:], sp, ts_[T][:])

        def s3me(T):
            img, j = divmod(T, NT)
            vs2 = vs2pool.tile([P, 2, SW], bf16, tag="vs2", name="vs2")
            vs2s[T] = vs2
            qC = ps2.tile([P, 2 * Wc], f32, tag="qC", name="qC")
            for s, ab in enumerate((aT, bT)):
                for c in range(NC_):
                    lo = c * CH
                    qlo = s * Wc + lo
                    nc.tensor.matmul(
                        qC[:, qlo : qlo + CH], w16sb[:, j, 0:128],
                        ab[T][:, lo : lo + CH],
                        start=True, stop=(j == 0 and j == NT - 1),
                    )
                    if j > 0:
                        nc.tensor.matmul(
                            qC[:, qlo : qlo + CH], w16sb[64:128, j, 128:256],
                            ab[T - 1][64:128, lo : lo + CH],
                            start=False, stop=(j == NT - 1),
                        )
                    if j < NT - 1:
                        nc.tensor.matmul(
                            qC[:, qlo : qlo + CH], w16sb[0:32, j, 256:384],
                            ab[T + 1][0:32, lo : lo + CH],
                            start=False, stop=True,
                        )
            nc.scalar.activation(vs2[:, :, 16 : 16 + Wc], qC[:], AF.Copy, scale=ALPHA)

        def s3s(T):
            vs2 = vs2s[T]
            mirrors(vs2)
            flat = vs2[:].rearrange("p s w -> p (s w)")
            L = 2 * SW
            init = cpool.tile([P, 1], f32, tag="init2", name="init2", bufs=2)
            nc.vector.reduce_sum(init[:], flat[:, 0:K], axis=AX)
            so2 = so2pool.tile([P, L], bf16, tag="so2", name="so2")
            nc.vector.tensor_tensor_scan(
                so2[:, 0 : L - K], flat[:, K:L], flat[:, 0 : L - K], init[:],
                op0=OP.add, op1=OP.subtract,
            )
            so2s[T] = so2

        def s4(T):
            img, j = divmod(T, NT)
            if j == 0:
                outBs[img] = opool.tile([P, NT, Wc], f32, tag="outB", name="outB")
            outB = outBs[img]
            xI8b = imgs[img][4]
            so2 = so2s[T]
            sa = so2[:, 0:Wc]
            sb = so2[:, SW : SW + Wc]
            o1 = cpool.tile([P, Wc], bf16, tag="o1", name="o1", bufs=2)
            eng = nc.vector if T >= NTOT - 3 else nc.gpsimd
            eng.tensor_mul(o1[:], sa, xI8b[:, j, :])
            eng.tensor_add(outB[:, j, :], sb, o1[:])
            nc.sync.dma_start(
                dout.ap()[img, :, j : j + 1, :], outB[:, j : j + 1, :]
            )

        s1me(0)
        s1me(1)
        s1s(0)
        for G in range(NTOT + 3):
            if G + 2 < NTOT:
                s1me(G + 2)
            if G < NTOT:
                s2a(G)
            if G + 1 < NTOT:
                s1s(G + 1)
            if G < NTOT:
                s2b(G)
            if 0 <= G - 3 < NTOT:
                s3s(G - 3)
            if 0 <= G - 3 < NTOT:
                s4(G - 3)
            if G < NTOT:
                s2c(G)
            if 0 <= G - 1 < NTOT:
                s2d(G - 1)
            if 0 <= G - 2 < NTOT:
                s3me(G - 2)

        for p_ in (ps2, ps1, cpool, so2pool, vs2pool, so1pool, vs1pool,
                   abpool, opool, xpool, wpool):
            p_.release()

    nc.compile()
    return nc


def _get_nc(n_img, Hc, Wc):
    key = (n_img, Hc, Wc)
    if key not in _CACHE:
        _CACHE[key] = build_nc(n_img, Hc, Wc)
    return _CACHE[key]


def _to_tiled(a, P=128):
    # [n, H, W] -> [n, P, NT, W] with row r = j*P + p stored at [p, j]
    n, H, W = a.shape
    return np.ascontiguousarray(a.reshape(n, H // P, P, W).transpose(0, 2, 1, 3))


def _from_tiled(a):
    # [n, P, NT, W] -> [n, H, W]
    n, P_, NT, W = a.shape
    return a.transpose(0, 2, 1, 3).reshape(n, NT * P_, W)


def kernel(guide, input_map):
    from concourse.bass_utils import run_bass_kernel_spmd

    B, C, Hc, Wc = guide.shape
    n_cores = 8
    n_img = B // n_cores
    NT = Hc // 128
    g = np.asarray(guide, dtype=np.float32).reshape(B, Hc, Wc)
    p = np.asarray(input_map, dtype=np.float32).reshape(B, Hc, Wc)

    f8 = ml_dtypes.float8_e4m3
    I8 = _to_tiled(g).astype(f8)
    p8 = _to_tiled(p).astype(f8)
    Ip8 = _to_tiled(g * p).astype(f8)
    II8 = _to_tiled(g * g).astype(f8)

    w8, _, w16 = _build_weights(Hc, NT)
    nc = _get_nc(n_img, Hc, Wc)
    in_maps = [
        {
            "I8": I8[i * n_img : (i + 1) * n_img],
            "p8": p8[i * n_img : (i + 1) * n_img],
            "Ip8": Ip8[i * n_img : (i + 1) * n_img],
            "II8": II8[i * n_img : (i + 1) * n_img],
            "w8": w8,
            "w16": w16,
        }
        for i in range(n_cores)
    ]
    res = run_bass_kernel_spmd(nc, in_maps, core_ids=list(range(n_cores)))
    out = np.concatenate(
        [_from_tiled(np.asarray(res.results[i]["out"])) for i in range(n_cores)], axis=0
    )
    return np.ascontiguousarray(out.reshape(B, C, Hc, Wc), dtype=np.float32)

